# revision 37
# baseline (speedup 1.0000x reference)
"""DCT-compressed attention (nn_DCTAttentionIdeal) on 8 Trainium2 NeuronCores.

Math (per head, reference ordering):
    S    = (Q*s) @ (K*mask*s)^T with s = D**-0.25             [N,N]
    atn  = softmax(S, axis=-1)
    Vd   = Qd @ (V*mask)                                      [M,D]
    out  = Qd^T @ ((Qd @ atn @ Qd^T) @ Vd)                    [N,D]

Kernel reshaping (exact in real arithmetic):
  - mask (and the s^2 = 1/8 score scale) folded on the host: K,V arrive
    pre-masked in bf16; 1/8 is the Exp activation's `scale`.
  - softmax max-subtraction skipped (scores ~N(0,1) after the 1/8 scale).
  - per-row 1/denom folded into DCT columns:
        A1^T[k,m] = sum_q exp(S)[q,k] * (Qd^T[q,m]/denom[q])
    so the [N,N] exp matrix is consumed unnormalized straight from SBUF.
  - tail fully reassociated to avoid the [M,M] DCT-space product:
        W = Qd^T @ Vd            [N,D]   (cheap: 2-step contractions)
        R = A1 @ W               [M,D]
        out = Qd^T_cols @ R      [N,D]
    which equals Qd^T (Qd atn Qd^T) (Qd V m) exactly.

All matmuls run bf16 (1 cycle/row, keeps the compiler's fast-weight-load
path enabled; fp32r operands disable FWL for the following matmul).
PSUM (8 banks) is allocated exactly: scores 2x[128,1024] (4), A1
pair-accumulators 2x[128,512] (2), R accumulator [128,512] (1), misc
Vd/W/out accumulator [128,512] (1).

The emission is a software pipeline over half-head "periods": while the
B-phase (A1 matmuls) of (head h, group g) runs, the A-phase (scores+exp)
of the next group interleaves 1:2, the R/out tail of previous heads and
the Vd/W prologue of the next head fill remaining PE slots.  The
softmax denominators ride the Exp activations' accumulators for the low
k-half and an (otherwise idle) GpSimd reduce for the high k-half.

Sharding: batch*heads (2*16=32) split 4-per-core across 8 cores; Q_dct
replicated; no cross-core communication.  Host pre-transposes/casts
(pure layout + dtype) and pre-applies the mask to K and V.
"""

import numpy as np
import ml_dtypes

import concourse.tile as tile
from concourse import bacc, mybir
from concourse import bass_utils

F32 = mybir.dt.float32
BF16 = mybir.dt.bfloat16
NPBF16 = ml_dtypes.bfloat16
AF = mybir.ActivationFunctionType
ALU = mybir.AluOpType
AX = mybir.AxisListType

B, H, N, D, M = 2, 16, 2048, 64, 256
NCORES = 8
HPC = (B * H) // NCORES  # heads per core = 4
NT = N // 128            # 16 (q and k 128-blocks)
MT = M // 128             # 2
P = 128


def _emit(tc, ctx, io):
    nc = tc.nc

    sh = ctx.enter_context(tc.tile_pool(name="shared", bufs=1))
    exp_pool = ctx.enter_context(tc.tile_pool(name="exp", bufs=16))
    cq_pool = ctx.enter_context(tc.tile_pool(name="cq", bufs=16))
    qt_pool = ctx.enter_context(tc.tile_pool(name="qt", bufs=2))
    kt_pool = ctx.enter_context(tc.tile_pool(name="kt", bufs=2))
    v_pool = ctx.enter_context(tc.tile_pool(name="v", bufs=4))
    a1_pool = ctx.enter_context(tc.tile_pool(name="a1", bufs=2))
    vd_pool = ctx.enter_context(tc.tile_pool(name="vd", bufs=4))
    w_pool = ctx.enter_context(tc.tile_pool(name="w", bufs=4))
    r_pool = ctx.enter_context(tc.tile_pool(name="r", bufs=2))
    ost_pool = ctx.enter_context(tc.tile_pool(name="ost", bufs=2))
    st_pool = ctx.enter_context(tc.tile_pool(name="st", bufs=8))

    ps_s = ctx.enter_context(tc.tile_pool(name="ps_s", bufs=2, space="PSUM"))
    ps_a1 = ctx.enter_context(tc.tile_pool(name="ps_a1", bufs=2, space="PSUM"))
    ps_r = ctx.enter_context(tc.tile_pool(name="ps_r", bufs=1, space="PSUM"))
    ps_x = ctx.enter_context(tc.tile_pool(name="ps_x", bufs=1, space="PSUM"))

    # --- shared tiles: Qd^T / Qd split into chunk-tiles so early consumers
    # only depend on the DMA chunk they actually read -------------------
    qdtr_a = sh.tile([P, 4, M], BF16)   # qdtr[p,t,m] = Qd[m, 128t+p], t<4
    qdtr_b1 = sh.tile([P, 4, M], BF16)  # t in 4..7
    qdtr_b2 = sh.tile([P, 8, M], BF16)  # t in 8..15
    qdtr_src = io["QdT"].rearrange("(t p) m -> p t m", p=P)
    # Qd tiles qdnr[p,c,q] = Qd[128c+p, q], split into 4 q-chunks of 512
    qdnr_c = [sh.tile([P, MT, 512], BF16, name=f"qdnr{i}") for i in range(4)]
    qdnr_src = io["Qd"].rearrange("(c p) q -> p c q", p=P)

    def qdtr_at(t):
        if t < 4:
            return qdtr_a, t
        if t < 8:
            return qdtr_b1, t - 4
        return qdtr_b2, t - 8

    def qdnr_lhsT(mh, k):
        # [128m, 128q] tile for q-block k, m-half mh
        return qdnr_c[k // 4][:, mh, (k % 4) * P : (k % 4 + 1) * P]

    ident = sh.tile([64, 64], F32)  # identity for PE transposes

    st = [dict() for _ in range(HPC)]

    def prep_dma(h):
        s = st[h]
        if h == 0:
            s["v"] = v_pool.tile([P, NT, D], BF16, name="v", tag="v")
            # startup-critical ordering; each chunk is its own tile so the
            # first score matmuls only wait for the bytes they read.  The
            # scalar engine's DGE queue has a much shorter preamble than
            # sync's, so it carries the first-needed chunks.
            s["qt_p"] = [
                qt_pool.tile([64, 1024], BF16, name="qt", tag=f"qt{i}")
                for i in range(2)
            ]
            s["kt_p"] = [
                kt_pool.tile([64, 512], BF16, name="kt", tag=f"kt{i}")
                for i in range(4)
            ]
            nc.scalar.dma_start(s["kt_p"][0][:], io["KT"][h][:, 0:512])
            nc.scalar.dma_start(s["qt_p"][0][:], io["QT"][h][:, 0:1024])
            nc.scalar.dma_start(qdtr_a[:], qdtr_src[:, 0:4, :])
            nc.scalar.dma_start(
                s["v"][:], io["V"][h].rearrange("(t p) d -> p t d", p=P)
            )
            for i in range(1, 4):
                nc.sync.dma_start(
                    s["kt_p"][i][:], io["KT"][h][:, i * 512 : (i + 1) * 512]
                )
            nc.sync.dma_start(qdtr_b1[:], qdtr_src[:, 4:8, :])
            nc.sync.dma_start(qdtr_b2[:], qdtr_src[:, 8:16, :])
            nc.sync.dma_start(s["qt_p"][1][:], io["QT"][h][:, 1024:2048])
            nc.sync.dma_start(qdnr_c[2][:], qdnr_src[:, :, 1024:1536])
            nc.sync.dma_start(qdnr_c[3][:], qdnr_src[:, :, 1536:2048])
            nc.sync.dma_start(ident[:], io["I64"])
            for h2 in range(1, HPC):
                st[h2]["v"] = v_pool.tile([P, NT, D], BF16, name="v", tag="v")
                nc.sync.dma_start(
                    st[h2]["v"][:], io["V"][h2].rearrange("(t p) d -> p t d", p=P)
                )
        else:
            qt = qt_pool.tile([64, N], BF16, name="qt", tag="qtw")
            kt = kt_pool.tile([64, N], BF16, name="kt", tag="ktw")
            nc.sync.dma_start(qt[:], io["QT"][h])
            nc.sync.dma_start(kt[:], io["KT"][h])
            s["qt_p"] = [qt]
            s["kt_p"] = [kt]
        s["exp"] = {}
        s["cq"] = {}
        s["a1"] = a1_pool.tile([P, NT, 2, M], BF16, name="a1", tag="a1")
        s["rstep"] = [0, 0]

    def qt_lhsT(s, qb):
        if len(s["qt_p"]) == 1:
            return s["qt_p"][0][:, qb * P : (qb + 1) * P]
        return s["qt_p"][qb // 8][:, (qb % 8) * P : (qb % 8 + 1) * P]

    def kt_rhs(s, lo):
        if len(s["kt_p"]) == 1:
            return s["kt_p"][0][:, lo : lo + 512]
        return s["kt_p"][lo // 512][:]

    def vd_unit(h, mh):
        # Vd[n,d] = sum_q Qd[n,q] (Vm)[q,d]; half n=[128mh:128mh+128]
        s = st[h]
        if mh == 0:
            s["vdps"] = ps_x.tile([P, 512], F32, name="px", tag="px")
            s["vd"] = vd_pool.tile([P, 2, D], BF16, name="vd", tag="vd")
        vps = s["vdps"]
        for t in range(NT):
            qd_t, tl = qdtr_at(t)
            nc.tensor.matmul(
                vps[:, mh * 64 : (mh + 1) * 64],
                lhsT=qd_t[:, tl, mh * P : (mh + 1) * P],
                rhs=s["v"][:, t, :],
                start=(t == 0),
                stop=(t == NT - 1),
            )
        if mh == 1:
            nc.vector.tensor_copy(s["vd"][:], vps[:, 0:128])

    def w_unit(h, j4):
        # W[k,d] = sum_n Qd[n,k] Vd[n,d]; k-tiles 4*j4 .. 4*j4+3
        s = st[h]
        if j4 == 0:
            s["w"] = w_pool.tile([P, NT, D], BF16, name="w", tag="w")
        wps = ps_x.tile([P, 512], F32, name="px", tag="px")
        for kk in range(4):
            k = j4 * 4 + kk
            for sg in range(2):
                nc.tensor.matmul(
                    wps[:, kk * 64 : (kk + 1) * 64],
                    lhsT=qdnr_lhsT(sg, k),
                    rhs=s["vd"][:, sg, :],
                    start=(sg == 0),
                    stop=(sg == 1),
                )
        nc.vector.tensor_copy(s["w"][:, j4 * 4 : (j4 + 1) * 4, :], wps[:, 0:256])

    def a_unit(h, qb):
        # scores + exp + denom + cq for q-block qb
        s = st[h]
        ex = exp_pool.tile([P, N], BF16, name="exp", tag="exp")
        s["exp"][qb] = ex
        sums = st_pool.tile([P, 1], F32, name="sums", tag="sums")
        red2 = st_pool.tile([P, 1], F32, name="red2", tag="red2")
        den = st_pool.tile([P, 1], F32, name="den", tag="den")
        rec = st_pool.tile([P, 1], F32, name="rec", tag="rec")
        cqt = cq_pool.tile([P, M], BF16, name="cq", tag="cq")
        s["cq"][qb] = cqt
        for c in range(2):
            sps = ps_s.tile([P, 1024], F32, name="s", tag="s")
            for j in range(2):
                lo = c * 1024 + j * 512
                nc.tensor.matmul(
                    sps[:, j * 512 : (j + 1) * 512],
                    lhsT=qt_lhsT(s, qb),
                    rhs=kt_rhs(s, lo),
                    start=True,
                    stop=True,
                )
            nc.scalar.activation(
                ex[:, c * 1024 : (c + 1) * 1024],
                sps[:],
                AF.Exp,
                scale=0.125,
                accum_out=(sums[:] if c == 0 else red2[:]),
            )
        nc.vector.tensor_add(den[:], sums[:], red2[:])
        nc.vector.reciprocal(rec[:], den[:])
        qd_t, tl = qdtr_at(qb)
        nc.vector.tensor_scalar_mul(cqt[:], qd_t[:, tl, :], rec[:])

    def b_unit(h, g, kc):
        # A1^T[k-block kc, m] += sum over group-g q-blocks
        s = st[h]
        if kc % 2 == 0:
            s["a1ps"] = ps_a1.tile([P, 512], F32, name="a1ps", tag="a1ps")
        aps = s["a1ps"]
        col = (kc % 2) * M
        for qi in range(8):
            qb = g * 8 + qi
            nc.tensor.matmul(
                aps[:, col : col + M],
                lhsT=s["exp"][qb][:, kc * P : (kc + 1) * P],
                rhs=s["cq"][qb][:],
                start=(qi == 0),
                stop=(qi == 7),
            )
        if kc % 2 == 1:
            nc.vector.tensor_copy(s["a1"][:, kc - 1 : kc + 1, g, :], aps[:])

    def r_steps(h, pairs):
        # R^T[d,m] += sum_k W[k,d] A1^T[k,m] over the given (kc, g) pairs.
        # W is the stationary side so the moving free dim is a fat 256 (the
        # [m,d]-direct orientation would emit 64-free matmuls whose weight
        # loads can't hide).
        s = st[h]
        if "rps" not in s:
            s["rps"] = ps_r.tile([64, M], F32, name="rps", tag="rps")
        for kc, g in pairs:
            i = s["rstep"][0]
            s["rstep"][0] = i + 1
            nc.tensor.matmul(
                s["rps"][:],
                lhsT=s["w"][:, kc, :],
                rhs=s["a1"][:, kc, g, :],
                start=(i == 0),
                stop=(i == 31),
            )

    def rt_copy(h):
        s = st[h]
        s["rt"] = r_pool.tile([64, M], F32, name="rt", tag="rt")
        nc.vector.tensor_copy(s["rt"][:], s["rps"][:])

    def r_transpose(h):
        # R^T [64d, 256m] -> R [128m, 2, 64d] via PE transpose (f32, reusing
        # the misc psum bank)
        s = st[h]
        tp = ps_x.tile([P, 512], F32, name="px", tag="px")
        for mh in range(2):
            nc.tensor.transpose(
                tp[:, mh * 64 : (mh + 1) * 64],
                s["rt"][:, mh * P : (mh + 1) * P],
                ident[:],
            )
        s["r"] = r_pool.tile([P, 2, D], BF16, name="r", tag="r")
        nc.vector.tensor_copy(s["r"][:], tp[:, 0:128])

    def out_unit(h, qb):
        # out[q-block qb, d] = sum_m Qd[m,q] R[m,d]; psum batched 4 q-blocks
        # per bank (sequential groups), one copy per batch
        s = st[h]
        if qb == 0:
            s["ost"] = ost_pool.tile([P, NT, D], F32, name="ost", tag="ost")
        if qb % 4 == 0:
            s["ops"] = ps_x.tile([P, 512], F32, name="px", tag="px")
        col = (qb % 4) * 64
        for mh in range(2):
            nc.tensor.matmul(
                s["ops"][:, col : col + 64],
                lhsT=qdnr_lhsT(mh, qb),
                rhs=s["r"][:, mh, :],
                start=(mh == 0),
                stop=(mh == 1),
            )
        if qb % 4 == 3:
            nc.vector.tensor_copy(
                s["ost"][:, qb - 3 : qb + 1, :], s["ops"][:, 0:256]
            )

    def out_dma(h, q0, q1):
        s = st[h]
        o_r = io["out"][h].rearrange("(t p) d -> p t d", p=P)
        nc.sync.dma_start(o_r[:, q0:q1, :], s["ost"][:, q0:q1, :])

    # --- software-pipelined emission ------------------------------------
    # prologue: pre-heat the PE clock with dummy matmuls while DMAs land,
    # then scores/exp of head-0 group-0 interleaved with Vd/W prologues
    # period h: [g0 slots kc=0..15][g1 slots kc=0..15]
    #   g0: B(h,0,kc); A(h, g1) every 2; R(h-1,mh1); Vd/W(2,3) in h=0;
    #       r_copy(h-1)
    #   g1: B(h,1,kc); A(h+1, g0) every 2; R(h,mh0); out(h-1)+dma(h-1)
    scr = sh.tile([P, 512], BF16)
    nc.vector.memset(scr[:], 1.0)
    prep_dma(0)
    heat = ps_x.tile([P, 512], F32, name="px", tag="px")
    for _ in range(6):
        nc.tensor.matmul(
            heat[:, 0:128], lhsT=scr[:, 0:128], rhs=scr[:, 0:128], start=True, stop=True
        )
    for qb in range(4):
        a_unit(0, qb)
    vd_unit(0, 0)
    a_unit(0, 4)
    nc.scalar.dma_start(qdnr_c[0][:], qdnr_src[:, :, 0:512])
    vd_unit(0, 1)
    a_unit(0, 5)
    nc.scalar.dma_start(qdnr_c[1][:], qdnr_src[:, :, 512:1024])
    w_unit(0, 0)
    w_unit(0, 1)
    a_unit(0, 6)
    w_unit(0, 2)
    w_unit(0, 3)
    a_unit(0, 7)
    vd_unit(1, 0)
    vd_unit(1, 1)
    for j4 in range(4):
        w_unit(1, j4)

    for h in range(HPC):
        for kc in range(NT):
            if kc == 0 and h + 1 < HPC:
                prep_dma(h + 1)
            b_unit(h, 0, kc)
            if kc == 0 and h >= 1:
                r_transpose(h - 1)
            if kc % 2 == 0:
                a_unit(h, 8 + kc // 2)
            if h >= 1:
                out_unit(h - 1, kc)
                if kc % 4 == 3:
                    out_dma(h - 1, kc - 3, kc + 1)
            if h == 0:
                if kc in (2, 3):
                    vd_unit(2, kc - 2)
                elif kc in (4, 5, 6, 7):
                    w_unit(2, kc - 4)
                elif kc in (8, 9):
                    vd_unit(3, kc - 8)
                elif kc in (10, 11, 12, 13):
                    w_unit(3, kc - 10)
        for kc in range(NT):
            b_unit(h, 1, kc)
            if kc % 2 == 0 and h + 1 < HPC:
                a_unit(h + 1, kc // 2)
            pairs = [(kc, 0)]
            if kc >= 2:
                pairs.append((kc - 2, 1))
            if kc == NT - 1:
                pairs += [(kc - 1, 1), (kc, 1)]
            r_steps(h, pairs)
        rt_copy(h)

    # drain: tail of last head
    hL = HPC - 1
    r_transpose(hL)
    for qb in range(NT):
        out_unit(hL, qb)
        if qb % 4 == 3:
            out_dma(hL, qb - 3, qb + 1)


def build_nc():
    from contextlib import ExitStack

    nc = bacc.Bacc("TRN2", target_bir_lowering=False, debug=False)
    io = {
        "QT": nc.dram_tensor("QT", [HPC, 64, N], BF16, kind="ExternalInput").ap(),
        "KT": nc.dram_tensor("KT", [HPC, 64, N], BF16, kind="ExternalInput").ap(),
        "V": nc.dram_tensor("V", [HPC, N, D], BF16, kind="ExternalInput").ap(),
        "QdT": nc.dram_tensor("QdT", [N, M], BF16, kind="ExternalInput").ap(),
        "Qd": nc.dram_tensor("Qd", [M, N], BF16, kind="ExternalInput").ap(),
        "I64": nc.dram_tensor("I64", [64, 64], F32, kind="ExternalInput").ap(),
        "out": nc.dram_tensor("out", [HPC, N, D], F32, kind="ExternalOutput").ap(),
    }
    with tile.TileContext(nc) as tc:
        with ExitStack() as ctx:
            _emit(tc, ctx, io)
    nc.compile()
    return nc


_NC = None


def _get_nc():
    global _NC
    if _NC is None:
        _NC = build_nc()
    return _NC


def make_in_maps(Q, K, V, mask, Q_dct):
    Q = np.asarray(Q, dtype=np.float32).reshape(B, H, N, D)
    K = np.asarray(K, dtype=np.float32).reshape(B, H, N, D)
    V = np.asarray(V, dtype=np.float32).reshape(B, H, N, D)
    mask = np.asarray(mask, dtype=np.float32)
    Q_dct = np.asarray(Q_dct, dtype=np.float32)

    m4 = mask[:, None, :, None]  # [B,1,N,1]
    Km = (K * m4).reshape(B * H, N, D)
    Vm = (V * m4).reshape(B * H, N, D).astype(NPBF16)
    Qf = Q.reshape(B * H, N, D)

    QT = np.ascontiguousarray(Qf.transpose(0, 2, 1)).astype(NPBF16)
    KT = np.ascontiguousarray(Km.transpose(0, 2, 1)).astype(NPBF16)
    QdT = np.ascontiguousarray(Q_dct.T).astype(NPBF16)
    Qd = np.ascontiguousarray(Q_dct).astype(NPBF16)
    I64 = np.eye(64, dtype=np.float32)

    in_maps = []
    for c in range(NCORES):
        sl = slice(HPC * c, HPC * (c + 1))
        in_maps.append(
            {
                "QT": np.ascontiguousarray(QT[sl]),
                "KT": np.ascontiguousarray(KT[sl]),
                "V": np.ascontiguousarray(Vm[sl]),
                "QdT": QdT,
                "Qd": Qd,
                "I64": I64,
            }
        )
    return in_maps


def run_on_device(in_maps, **kwargs):
    nc = _get_nc()
    return bass_utils.run_bass_kernel_spmd(
        nc, in_maps, core_ids=list(range(NCORES)), **kwargs
    )


def kernel(Q, K, V, mask, Q_dct):
    in_maps = make_in_maps(Q, K, V, mask, Q_dct)
    res = run_on_device(in_maps)
    out = np.empty((B * H, N, D), dtype=np.float32)
    for c in range(NCORES):
        out[HPC * c : HPC * (c + 1)] = res.results[c]["out"]
    return out.reshape(B, H, N, D)
